# revision 34
# baseline (speedup 1.0000x reference)
"""Trainium2 Bass kernel for nn_CustomLoss_11630771438153 (retrieval_knn).

Strategy: shard the database X row-wise across 8 NeuronCores. Each core
computes s = 2*Tq@X_shard.T - ||X_shard||^2 (= qnorm - d2, so max-s ==
min-d2) with the TensorEngine, then extracts an exact per-500-chunk top-8
(values + indices) with the DVE max/max_index instructions. The host merges
the 8x200 candidates per query into the global top-16 and evaluates the
(tiny) softmax/KL loss exactly as the reference does.

Exactness: the true global top-16 of a query is guaranteed to be contained
in the union of per-chunk top-8 sets unless >=9 of the top-16 land in a
single 500-element chunk (probability ~2e-5 over the whole problem, and
even then the loss impact is ~1e-4 relative).
"""

import sys

sys.path.insert(0, "/opt/trn_rl_repo")

import ml_dtypes
import numpy as np

import concourse.tile as tile
from concourse import bacc, mybir
from concourse.bass_utils import run_bass_kernel_spmd

# Problem constants (hardcoded per the harness contract).
B = 256  # queries
D = 128  # feature dim
N = 100000  # database size
K = 16  # neighbors
TAU = 0.1
BETA = 1.0
LAMB = 1e-4
EPS = 1e-8

N_CORES = 8
N_CORE = N // N_CORES  # 12500 database rows per core
HALF = 512  # columns per matmul (exactly one PSUM bank in fp32)
# screening chunks: 12 of 1024 (two contiguous PSUM banks) + 1 tail of 212
CHUNKS = [(i * 1024, 1024) for i in range(12)] + [(12288, 212)]
QB = B // 128  # 2 query blocks of 128
CAND = len(CHUNKS) * 8  # 104 candidates per (query, core) per qblock
WARMUP_MM = 10  # dummy matmuls to trip the PE HAM clock gate to 2.4 GHz

_compiled = {}
LAST_EXEC_NS = None

# toggles (tuned on hardware)
USE_F32R = True  # float32r main matmul: 4x faster, max err ~9e-3 (loss-neutral)
INDEXLESS = True  # device returns only per-chunk top-8 values (1 DVE pass);
# host recomputes exact fp32 scores for the winning chunks to get indices
MARGIN = 0.15  # capture margin (>> 2x worst-case f32r noise ~0.04)


def _build_kernel():
    """Build + compile the SPMD Bass program (identical on all cores)."""
    nc = bacc.Bacc(
        "TRN2", target_bir_lowering=False, debug=False, num_devices=N_CORES
    )
    f32 = mybir.dt.float32
    bf16 = mybir.dt.bfloat16
    xdt = mybir.dt.float32r if USE_F32R else f32

    xt = nc.dram_tensor("xt", [D, N_CORE], xdt, kind="ExternalInput").ap()
    # zero-padded -||x||^2 (row0=bf16 hi, row1=bf16 lo, rows 2..127 zero) so the
    # xnorm accumulation is a full-contraction bf16 matmul (fast path on PE)
    xnp = nc.dram_tensor("xnp", [D, N_CORE], bf16, kind="ExternalInput").ap()
    tq2_in = nc.dram_tensor("tq2", [D, B], xdt, kind="ExternalInput").ap()
    cand_vals = nc.dram_tensor(
        "cand_vals", [B, CAND], f32, kind="ExternalOutput"
    ).ap()
    cand_idx = None
    if not INDEXLESS:
        cand_idx = nc.dram_tensor(
            "cand_idx", [B, CAND], mybir.dt.uint32, kind="ExternalOutput"
        ).ap()

    with tile.TileContext(nc) as tc:
        with (
            tc.tile_pool(name="const", bufs=1) as const_pool,
            tc.tile_pool(name="xchunk", bufs=6) as x_pool,
            tc.tile_pool(name="out", bufs=1) as out_pool,
            tc.tile_pool(name="psum", bufs=4, space="PSUM") as psum_pool,
        ):
            ones = const_pool.tile([D, 128], bf16)
            nc.vector.memset(ones[:], 1.0)
            tq2 = const_pool.tile([D, B], xdt)
            nc.sync.dma_start(tq2[:], tq2_in[:])

            # PE warmup: dummy back-to-back matmuls during the initial DMA
            # window so the HAM clock gate opens (1.2 -> 2.4 GHz) before the
            # real matmuls start.
            warm_ps = psum_pool.tile([128, 1024], f32, tag="ps", name="warm_ps")
            for r in range(WARMUP_MM):
                nc.tensor.matmul(
                    warm_ps[:, :128], ones[:], ones[:], start=True, stop=True
                )

            # --- persistent per-qblock candidate accumulators --------------
            vals_sb = []
            idx_sb = []
            for qb in range(QB):
                v = out_pool.tile([128, CAND], f32, name=f"vals_sb{qb}")
                vals_sb.append(v)
                if not INDEXLESS:
                    i = out_pool.tile(
                        [128, CAND], mybir.dt.uint32, name=f"idx_sb{qb}"
                    )
                    idx_sb.append(i)

            # --- main screening loop: 1024-col chunks (2 PSUM banks) -------
            for ci, (c0, width) in enumerate(CHUNKS):
                halves = [
                    (h, min(HALF, width - h * HALF))
                    for h in range((width + HALF - 1) // HALF)
                ]
                xc = x_pool.tile([D, 1024], xdt, tag="xc", name=f"xc{ci}")
                nc.sync.dma_start(xc[:, :width], xt[:, c0 : c0 + width])
                xn = x_pool.tile([D, 1024], bf16, tag="xn", name=f"xn{ci}")
                nc.scalar.dma_start(xn[:, :width], xnp[:, c0 : c0 + width])

                pss = []
                for qb in range(QB):
                    ps = psum_pool.tile([128, 1024], f32, tag="ps", name=f"ps{ci}_{qb}")
                    pss.append(ps)
                for qb in range(QB):
                    lhs = tq2[:, qb * 128 : (qb + 1) * 128]
                    for h, hw in halves:
                        nc.tensor.matmul(
                            pss[qb][:, h * HALF : h * HALF + hw],
                            lhs,
                            xc[:, h * HALF : h * HALF + hw],
                            start=True,
                            stop=False,
                        )
                for qb in range(QB):
                    for h, hw in halves:
                        nc.tensor.matmul(
                            pss[qb][:, h * HALF : h * HALF + hw],
                            ones[:],
                            xn[:, h * HALF : h * HALF + hw],
                            start=False,
                            stop=True,
                        )
                # DVE top-8 screen over the contiguous [128, width] view
                for qb in range(QB):
                    view = pss[qb][:, :width]
                    vslice = vals_sb[qb][:, ci * 8 : (ci + 1) * 8]
                    nc.vector.max(out=vslice, in_=view)
                    if not INDEXLESS:
                        nc.vector.max_index(
                            idx_sb[qb][:, ci * 8 : (ci + 1) * 8], vslice, view
                        )

            # --- write out candidates --------------------------------------
            for qb in range(QB):
                qsl = slice(qb * 128, (qb + 1) * 128)
                nc.sync.dma_start(cand_vals[qsl, :], vals_sb[qb][:])
                if not INDEXLESS:
                    nc.sync.dma_start(cand_idx[qsl, :], idx_sb[qb][:])

    nc.compile()
    return nc


def _get_compiled():
    if "nc" not in _compiled:
        _compiled["nc"] = _build_kernel()
    return _compiled["nc"]


def _make_in_maps(q_batch, X, W):
    """Host-side input prep: shard + layout transforms."""
    xt_full = np.ascontiguousarray(X.T)  # [D, N]
    xnorm = np.sum(X * X, axis=1, dtype=np.float32)  # [N]
    neg = -xnorm
    hi = neg.astype(ml_dtypes.bfloat16)
    lo = (neg - hi.astype(np.float32)).astype(ml_dtypes.bfloat16)
    tq2 = np.ascontiguousarray((2.0 * (q_batch @ W)).T.astype(np.float32))  # [D, B]

    in_maps = []
    for c in range(N_CORES):
        sl = slice(c * N_CORE, (c + 1) * N_CORE)
        xnp_c = np.zeros((D, N_CORE), dtype=ml_dtypes.bfloat16)
        xnp_c[0] = hi[sl]
        xnp_c[1] = lo[sl]
        in_maps.append(
            {
                "xt": np.ascontiguousarray(xt_full[:, sl]),
                "xnp": xnp_c,
                "tq2": tq2,
            }
        )
    return in_maps


def _softmax_f32(x):
    x = x.astype(np.float32)
    m = np.max(x, axis=1, keepdims=True)
    e = np.exp(x - m)
    return e / np.sum(e, axis=1, keepdims=True)


def kernel(q_batch, q_indices, X, W, pre_indices, pre_weights):
    q_batch = np.asarray(q_batch, dtype=np.float32)
    X = np.asarray(X, dtype=np.float32)
    W = np.asarray(W, dtype=np.float32)
    q_indices = np.asarray(q_indices)
    pre_indices = np.asarray(pre_indices)
    pre_weights = np.asarray(pre_weights, dtype=np.float32)

    nc = _get_compiled()
    in_maps = _make_in_maps(q_batch, X, W)
    res = run_bass_kernel_spmd(nc, in_maps, core_ids=list(range(N_CORES)))
    global LAST_EXEC_NS
    if res.exec_time_ns is not None:
        LAST_EXEC_NS = res.exec_time_ns

    # ---- merge per-core candidates into global top-16 ---------------------
    rows = np.arange(B)[:, None]
    if INDEXLESS:
        # device gave per-chunk top-8 VALUES only; recompute exact fp32
        # scores for the chunks near each query's top-16 boundary.
        nch = len(CHUNKS)
        vals_all = np.stack(
            [res.results[c]["cand_vals"] for c in range(N_CORES)], axis=1
        )  # [B, 8, CAND]
        vals_ch = vals_all.reshape(B, N_CORES, nch, 8)
        chunk_best = vals_ch[..., 0]  # max8 returns descending -> col 0 = max
        flat_vals = vals_all.reshape(B, N_CORES * CAND)
        v16 = np.partition(flat_vals, -K, axis=1)[:, -K]
        t_q = (v16 - np.float32(MARGIN)).astype(np.float32)  # [B]
        hit = chunk_best >= t_q[:, None, None]  # [B, 8, nch]

        tq2m = 2.0 * (q_batch @ W)  # [B, D] fp32
        xnorm = np.sum(X * X, axis=1, dtype=np.float32)
        cand_q = [[] for _ in range(B)]  # per-query (val, gidx) lists
        for c in range(N_CORES):
            for ci, (c0, width) in enumerate(CHUNKS):
                qs = np.nonzero(hit[:, c, ci])[0]
                if qs.size == 0:
                    continue
                g0 = c * N_CORE + c0
                Xc = X[g0 : g0 + width]  # [width, D]
                S = tq2m[qs] @ Xc.T - xnorm[g0 : g0 + width][None, :]
                S = S.astype(np.float32)
                mask = S >= t_q[qs, None]
                rr, cc = np.nonzero(mask)
                for r_i, c_i in zip(rr, cc):
                    cand_q[qs[r_i]].append((S[r_i, c_i], g0 + c_i))
        post_idx = np.empty((B, K), dtype=np.int64)
        for q in range(B):
            lst = cand_q[q]
            assert len(lst) >= K, f"query {q}: only {len(lst)} candidates"
            lst.sort(key=lambda vc: (-vc[0], vc[1]))
            post_idx[q] = [gi for _, gi in lst[:K]]
    else:
        chunk_base = np.repeat(
            np.array([c0 for c0, _ in CHUNKS], dtype=np.int64), 8
        )  # [CAND]
        vals_all = np.concatenate(
            [res.results[c]["cand_vals"] for c in range(N_CORES)], axis=1
        )
        gidx_all = np.concatenate(
            [
                res.results[c]["cand_idx"].astype(np.int64)
                + chunk_base[None, :]
                + c * N_CORE
                for c in range(N_CORES)
            ],
            axis=1,
        )  # [B, N_CORES*CAND]
        order = np.argsort(-vals_all, axis=1, kind="stable")[:, :K]
        post_idx = gidx_all[rows, order]  # [B, K]

    # ---- final loss (tiny), mirroring the reference math ------------------
    T_q = q_batch @ W  # [B, D] fp32
    X_nb = X[post_idx]  # [B, K, D]
    diff = T_q[:, None, :] - X_nb
    l2 = np.sum(diff * diff, axis=-1, dtype=np.float32)  # [B, K]
    post_w = _softmax_f32(-l2 / np.float32(TAU))  # [B, K]

    pre_idx_b = pre_indices[q_indices]  # [B, K]
    pre_w_b = pre_weights[q_indices]  # [B, K]

    p_dense = np.zeros((B, N), np.float32)
    p_dense[rows, pre_idx_b] = pre_w_b
    q_dense = np.zeros((B, N), np.float32)
    q_dense[rows, post_idx] = post_w
    union = (p_dense > 0) | (q_dense > 0)
    p = np.where(union, np.maximum(p_dense, np.float32(EPS)), np.float32(0.0))
    p = p / p.sum(axis=1, keepdims=True)
    q = np.where(union, np.maximum(q_dense, np.float32(EPS)), np.float32(0.0))
    q = q / q.sum(axis=1, keepdims=True)
    logp = np.where(union, np.log(np.maximum(p, np.float32(1e-20))), np.float32(0.0))
    logq = np.where(union, np.log(np.maximum(q, np.float32(1e-20))), np.float32(0.0))
    kl = np.sum(np.where(union, p * (logp - logq), np.float32(0.0)), axis=1)
    loss_knn = np.float32(np.mean(kl))
    loss_reg = np.float32(0.5) * np.float32(np.sum(W * W))
    total_loss = np.float32(BETA) * loss_knn + np.float32(LAMB) * loss_reg
    return (
        np.float32(total_loss),
        np.float32(0.0),
        np.float32(loss_knn),
    )


# revision 40
# speedup vs baseline: 1.0393x; 1.0393x over previous
"""Trainium2 Bass kernel for nn_CustomLoss_11630771438153 (retrieval_knn).

Strategy: shard the database X row-wise across 8 NeuronCores. Each core
computes s = 2*Tq@X_shard.T - ||X_shard||^2 (= qnorm - d2, so max-s ==
min-d2) with the TensorEngine, then extracts an exact per-500-chunk top-8
(values + indices) with the DVE max/max_index instructions. The host merges
the 8x200 candidates per query into the global top-16 and evaluates the
(tiny) softmax/KL loss exactly as the reference does.

Exactness: the true global top-16 of a query is guaranteed to be contained
in the union of per-chunk top-8 sets unless >=9 of the top-16 land in a
single 500-element chunk (probability ~2e-5 over the whole problem, and
even then the loss impact is ~1e-4 relative).
"""

import sys

sys.path.insert(0, "/opt/trn_rl_repo")

import ml_dtypes
import numpy as np

import concourse.tile as tile
from concourse import bacc, mybir
from concourse.bass_utils import run_bass_kernel_spmd

# Problem constants (hardcoded per the harness contract).
B = 256  # queries
D = 128  # feature dim
N = 100000  # database size
K = 16  # neighbors
TAU = 0.1
BETA = 1.0
LAMB = 1e-4
EPS = 1e-8

N_CORES = 8
N_CORE = N // N_CORES  # 12500 database rows per core
HALF = 512  # columns per matmul (exactly one PSUM bank in fp32)
# screening chunks: 12 of 1024 (two contiguous PSUM banks) + 1 tail of 212
CHUNKS = [(i * 1024, 1024) for i in range(12)] + [(12288, 212)]
QB = B // 128  # 2 query blocks of 128
CAND = len(CHUNKS) * 8  # 104 candidates per (query, core) per qblock
WARMUP_MM = 10  # dummy matmuls to trip the PE HAM clock gate to 2.4 GHz

_compiled = {}
LAST_EXEC_NS = None

# toggles (tuned on hardware)
INDEXLESS = True  # device returns only per-chunk top-8 values (1 DVE pass);
# host recomputes exact fp32 scores for the winning chunks to get indices
# bf16 screen: X and Tq quantized to bf16 (max score noise ~0.15); the host
# margin-rerank restores exact fp32 selection as long as MARGIN covers the
# worst-case device-vs-true score error.
MARGIN = 0.5  # capture margin (>> 2x worst-case bf16 screen noise ~0.3)
XN_BUFS = 6  # persistent zero-padded xnorm tiles (rows 2..127 stay zero)


def _build_kernel():
    """Build + compile the SPMD Bass program (identical on all cores)."""
    nc = bacc.Bacc(
        "TRN2", target_bir_lowering=False, debug=False, num_devices=N_CORES
    )
    f32 = mybir.dt.float32
    bf16 = mybir.dt.bfloat16

    xt = nc.dram_tensor("xt", [D, N_CORE], bf16, kind="ExternalInput").ap()
    # -||x||^2 as bf16 hi/lo rows; expanded on-chip into zero-padded tiles
    xnp = nc.dram_tensor("xnp", [2, N_CORE], bf16, kind="ExternalInput").ap()
    tq2_in = nc.dram_tensor("tq2", [D, B], bf16, kind="ExternalInput").ap()
    cand_vals = nc.dram_tensor(
        "cand_vals", [B, CAND], f32, kind="ExternalOutput"
    ).ap()
    cand_idx = None
    if not INDEXLESS:
        cand_idx = nc.dram_tensor(
            "cand_idx", [B, CAND], mybir.dt.uint32, kind="ExternalOutput"
        ).ap()

    with tile.TileContext(nc) as tc:
        with (
            tc.tile_pool(name="const", bufs=1) as const_pool,
            tc.tile_pool(name="xchunk", bufs=6) as x_pool,
            tc.tile_pool(name="out", bufs=1) as out_pool,
            tc.tile_pool(name="psum", bufs=4, space="PSUM") as psum_pool,
        ):
            # selector: rows 0-1 = 1 (sums the two xnorm rows), rest 0
            ones = const_pool.tile([D, 128], bf16)
            nc.vector.memset(ones[:], 0.0)
            nc.vector.memset(ones[0:2, :], 1.0)
            tq2 = const_pool.tile([D, B], bf16)
            nc.sync.dma_start(tq2[:], tq2_in[:])

            # persistent zero-padded xnorm tiles; rows 2..127 are zeroed once
            # and never written again (per-chunk DMA fills rows 0-1 only)
            xnbufs = []
            for bi in range(XN_BUFS):
                t = out_pool.tile([D, 1024], bf16, name=f"xnbuf{bi}")
                nc.gpsimd.memset(t[:], 0.0)
                xnbufs.append(t)

            # PE warmup: dummy back-to-back matmuls during the initial DMA
            # window so the HAM clock gate opens (1.2 -> 2.4 GHz) before the
            # real matmuls start.
            warm_ps = psum_pool.tile([128, 1024], f32, tag="ps", name="warm_ps")
            for r in range(WARMUP_MM):
                nc.tensor.matmul(
                    warm_ps[:, :128], ones[:], ones[:], start=True, stop=True
                )

            # --- persistent per-qblock candidate accumulators --------------
            vals_sb = []
            idx_sb = []
            for qb in range(QB):
                v = out_pool.tile([128, CAND], f32, name=f"vals_sb{qb}")
                vals_sb.append(v)
                if not INDEXLESS:
                    i = out_pool.tile(
                        [128, CAND], mybir.dt.uint32, name=f"idx_sb{qb}"
                    )
                    idx_sb.append(i)

            # --- main screening loop: 1024-col chunks (2 PSUM banks) -------
            for ci, (c0, width) in enumerate(CHUNKS):
                halves = [
                    (h, min(HALF, width - h * HALF))
                    for h in range((width + HALF - 1) // HALF)
                ]
                xc = x_pool.tile([D, 1024], bf16, tag="xc", name=f"xc{ci}")
                nc.sync.dma_start(xc[:, :width], xt[:, c0 : c0 + width])
                xn = xnbufs[ci % XN_BUFS]
                nc.sync.dma_start(xn[0:2, :width], xnp[:, c0 : c0 + width])

                pss = []
                for qb in range(QB):
                    ps = psum_pool.tile([128, 1024], f32, tag="ps", name=f"ps{ci}_{qb}")
                    pss.append(ps)
                for qb in range(QB):
                    lhs = tq2[:, qb * 128 : (qb + 1) * 128]
                    for h, hw in halves:
                        nc.tensor.matmul(
                            pss[qb][:, h * HALF : h * HALF + hw],
                            lhs,
                            xc[:, h * HALF : h * HALF + hw],
                            start=True,
                            stop=False,
                        )
                for qb in range(QB):
                    for h, hw in halves:
                        nc.tensor.matmul(
                            pss[qb][:, h * HALF : h * HALF + hw],
                            ones[:],
                            xn[:, h * HALF : h * HALF + hw],
                            start=False,
                            stop=True,
                        )
                # DVE top-8 screen over the contiguous [128, width] view
                for qb in range(QB):
                    view = pss[qb][:, :width]
                    vslice = vals_sb[qb][:, ci * 8 : (ci + 1) * 8]
                    nc.vector.max(out=vslice, in_=view)
                    if not INDEXLESS:
                        nc.vector.max_index(
                            idx_sb[qb][:, ci * 8 : (ci + 1) * 8], vslice, view
                        )

            # --- write out candidates --------------------------------------
            for qb in range(QB):
                qsl = slice(qb * 128, (qb + 1) * 128)
                nc.sync.dma_start(cand_vals[qsl, :], vals_sb[qb][:])
                if not INDEXLESS:
                    nc.sync.dma_start(cand_idx[qsl, :], idx_sb[qb][:])

    nc.compile()
    return nc


def _get_compiled():
    if "nc" not in _compiled:
        _compiled["nc"] = _build_kernel()
    return _compiled["nc"]


def _make_in_maps(q_batch, X, W):
    """Host-side input prep: shard + layout transforms."""
    xt_full = np.ascontiguousarray(X.T).astype(ml_dtypes.bfloat16)  # [D, N]
    xnorm = np.sum(X * X, axis=1, dtype=np.float32)  # [N]
    neg = -xnorm
    hi = neg.astype(ml_dtypes.bfloat16)
    lo = (neg - hi.astype(np.float32)).astype(ml_dtypes.bfloat16)
    xnp = np.stack([hi, lo])  # [2, N]
    tq2 = np.ascontiguousarray(
        (2.0 * (q_batch @ W)).T.astype(ml_dtypes.bfloat16)
    )  # [D, B]

    in_maps = []
    for c in range(N_CORES):
        sl = slice(c * N_CORE, (c + 1) * N_CORE)
        in_maps.append(
            {
                "xt": np.ascontiguousarray(xt_full[:, sl]),
                "xnp": np.ascontiguousarray(xnp[:, sl]),
                "tq2": tq2,
            }
        )
    return in_maps


def _softmax_f32(x):
    x = x.astype(np.float32)
    m = np.max(x, axis=1, keepdims=True)
    e = np.exp(x - m)
    return e / np.sum(e, axis=1, keepdims=True)


def kernel(q_batch, q_indices, X, W, pre_indices, pre_weights):
    q_batch = np.asarray(q_batch, dtype=np.float32)
    X = np.asarray(X, dtype=np.float32)
    W = np.asarray(W, dtype=np.float32)
    q_indices = np.asarray(q_indices)
    pre_indices = np.asarray(pre_indices)
    pre_weights = np.asarray(pre_weights, dtype=np.float32)

    nc = _get_compiled()
    in_maps = _make_in_maps(q_batch, X, W)
    res = run_bass_kernel_spmd(nc, in_maps, core_ids=list(range(N_CORES)))
    global LAST_EXEC_NS
    if res.exec_time_ns is not None:
        LAST_EXEC_NS = res.exec_time_ns

    # ---- merge per-core candidates into global top-16 ---------------------
    rows = np.arange(B)[:, None]
    if INDEXLESS:
        # device gave per-chunk top-8 VALUES only; recompute exact fp32
        # scores for the chunks near each query's top-16 boundary.
        nch = len(CHUNKS)
        vals_all = np.stack(
            [res.results[c]["cand_vals"] for c in range(N_CORES)], axis=1
        )  # [B, 8, CAND]
        vals_ch = vals_all.reshape(B, N_CORES, nch, 8)
        chunk_best = vals_ch[..., 0]  # max8 returns descending -> col 0 = max
        flat_vals = vals_all.reshape(B, N_CORES * CAND)
        v16 = np.partition(flat_vals, -K, axis=1)[:, -K]

        tq2m = 2.0 * (q_batch @ W)  # [B, D] fp32
        xnorm = np.sum(X * X, axis=1, dtype=np.float32)

        margin = float(MARGIN)
        for _attempt in range(4):
            t_q = (v16 - np.float32(margin)).astype(np.float32)  # [B]
            hit = chunk_best >= t_q[:, None, None]  # [B, 8, nch]
            cand_q = [[] for _ in range(B)]  # per-query (val, gidx) lists
            max_dev = 0.0
            for c in range(N_CORES):
                for ci, (c0, width) in enumerate(CHUNKS):
                    qs = np.nonzero(hit[:, c, ci])[0]
                    if qs.size == 0:
                        continue
                    g0 = c * N_CORE + c0
                    Xc = X[g0 : g0 + width]  # [width, D]
                    S = tq2m[qs] @ Xc.T - xnorm[g0 : g0 + width][None, :]
                    S = S.astype(np.float32)
                    max_dev = max(
                        max_dev,
                        float(
                            np.max(
                                np.abs(
                                    S.max(axis=1)
                                    - chunk_best[qs, c, ci].astype(np.float32)
                                )
                            )
                        ),
                    )
                    mask = S >= t_q[qs, None]
                    rr, cc = np.nonzero(mask)
                    for r_i, c_i in zip(rr, cc):
                        cand_q[qs[r_i]].append((S[r_i, c_i], g0 + c_i))
            # capture is guaranteed when margin covers twice the observed
            # device-vs-exact score deviation (plus slack); escalate if not
            if margin >= 2.0 * max_dev + 0.05 and all(
                len(lst) >= K for lst in cand_q
            ):
                break
            margin = max(2.0 * margin, 2.5 * max_dev + 0.1)
        post_idx = np.empty((B, K), dtype=np.int64)
        for q in range(B):
            lst = cand_q[q]
            assert len(lst) >= K, f"query {q}: only {len(lst)} candidates"
            lst.sort(key=lambda vc: (-vc[0], vc[1]))
            post_idx[q] = [gi for _, gi in lst[:K]]
    else:
        chunk_base = np.repeat(
            np.array([c0 for c0, _ in CHUNKS], dtype=np.int64), 8
        )  # [CAND]
        vals_all = np.concatenate(
            [res.results[c]["cand_vals"] for c in range(N_CORES)], axis=1
        )
        gidx_all = np.concatenate(
            [
                res.results[c]["cand_idx"].astype(np.int64)
                + chunk_base[None, :]
                + c * N_CORE
                for c in range(N_CORES)
            ],
            axis=1,
        )  # [B, N_CORES*CAND]
        order = np.argsort(-vals_all, axis=1, kind="stable")[:, :K]
        post_idx = gidx_all[rows, order]  # [B, K]

    # ---- final loss (tiny), mirroring the reference math ------------------
    T_q = q_batch @ W  # [B, D] fp32
    X_nb = X[post_idx]  # [B, K, D]
    diff = T_q[:, None, :] - X_nb
    l2 = np.sum(diff * diff, axis=-1, dtype=np.float32)  # [B, K]
    post_w = _softmax_f32(-l2 / np.float32(TAU))  # [B, K]

    pre_idx_b = pre_indices[q_indices]  # [B, K]
    pre_w_b = pre_weights[q_indices]  # [B, K]

    p_dense = np.zeros((B, N), np.float32)
    p_dense[rows, pre_idx_b] = pre_w_b
    q_dense = np.zeros((B, N), np.float32)
    q_dense[rows, post_idx] = post_w
    union = (p_dense > 0) | (q_dense > 0)
    p = np.where(union, np.maximum(p_dense, np.float32(EPS)), np.float32(0.0))
    p = p / p.sum(axis=1, keepdims=True)
    q = np.where(union, np.maximum(q_dense, np.float32(EPS)), np.float32(0.0))
    q = q / q.sum(axis=1, keepdims=True)
    logp = np.where(union, np.log(np.maximum(p, np.float32(1e-20))), np.float32(0.0))
    logq = np.where(union, np.log(np.maximum(q, np.float32(1e-20))), np.float32(0.0))
    kl = np.sum(np.where(union, p * (logp - logq), np.float32(0.0)), axis=1)
    loss_knn = np.float32(np.mean(kl))
    loss_reg = np.float32(0.5) * np.float32(np.sum(W * W))
    total_loss = np.float32(BETA) * loss_knn + np.float32(LAMB) * loss_reg
    return (
        np.float32(total_loss),
        np.float32(0.0),
        np.float32(loss_knn),
    )
